# revision 28
# baseline (speedup 1.0000x reference)
"""AlibiCausalSelfAttention on 8 Trainium2 NeuronCores — v2.

Sharding: data-parallel over batch (B=2) x head-parallel over head groups
(16 heads -> 4 groups of 4). Core c handles batch c//4, heads [4*(c%4), 4*(c%4)+4).
Each core computes a partial projection output [T, C] fp16 (W_proj row-sharded);
the host sums the 4 partials per batch in fp32 and adds b_proj.

v2 structural changes vs v1 (221us):
  - qk bias folded into the projection matmul as a 9th K=1 accumulation step
    (lhsT = bias row, rhs = ones row); PSUM->SBUF evacuations become pure
    cast-copies: qk on DVE, v on ACT (idle during phase 1).
  - v tiles store [64 v-dims | 64 ones] per head, so the PV matmul (M=128)
    broadcasts the softmax denominator into PSUM rows 64..127 for free.
    Normalization is then just reciprocal_approx_fast (rows 64:128 -> 0:64
    partition shift) + one tensor_mul. Replaces v1's reciprocal/broadcast-
    matmul/copy chain (~58us DVE -> ~22us).
  - St pair matmuls placed at packed column offsets [0:Wa|Wa:Wa+Wb] so exp
    covers one contiguous trimmed range; exp writes fp16 directly.  Diagonal
    causal masking via a single tensor_tensor MIN against a {30000,0} mask
    (also squashes fp16 exp overflow inf -> 0).
  - Software-pipelined St(n+1) before PV(n) so the in-order PE queue never
    stalls on ACT's exp.
  - Single tile scope; program order interleaves phase 1 (heads 0,1 first),
    attention, and output projection so all engines stay busy and the PE
    HAM clock gate stays warm.
  - fp16 output (halves the store DMA).
"""

import sys

if "/opt/trn_rl_repo" not in sys.path:
    sys.path.insert(0, "/opt/trn_rl_repo")

import numpy as np

T = 2048
C = 1024
H = 16
D = 64
HL = 4          # heads per core
HD = HL * D     # 256 local head dims
IW = 512        # i-tile width

_CACHE = {}


def _build_nc():
    import concourse.mybir as mybir
    import concourse.tile as tile
    from concourse import bacc
    from contextlib import ExitStack

    f32 = mybir.dt.float32
    fr = mybir.dt.float16
    Exp = mybir.ActivationFunctionType.Exp
    Min = mybir.AluOpType.min

    nc = bacc.Bacc("TRN2", target_bir_lowering=False, debug=False, num_devices=8)

    xT = nc.dram_tensor("xT", [C, T], fr, kind="ExternalInput").ap()
    wqk = nc.dram_tensor("wqk", [C, 2 * HD], fr, kind="ExternalInput").ap()
    bqk = nc.dram_tensor("bqk", [1, 2 * HD], fr, kind="ExternalInput").ap()
    wv = nc.dram_tensor("wv", [C, HD], fr, kind="ExternalInput").ap()
    bv = nc.dram_tensor("bv", [1, HD], fr, kind="ExternalInput").ap()
    wp = nc.dram_tensor("wp", [HD, C], fr, kind="ExternalInput").ap()
    aq = nc.dram_tensor("aq", [2 * HL, T], fr, kind="ExternalInput").ap()
    ak = nc.dram_tensor("ak", [2, T], fr, kind="ExternalInput").ap()
    umin_d = nc.dram_tensor("umin", [128, 128], fr, kind="ExternalInput").ap()
    onesr_d = nc.dram_tensor("onesr", [1, IW], fr, kind="ExternalInput").ap()
    out = nc.dram_tensor("out", [T, C], fr, kind="ExternalOutput").ap()

    NT16 = T // 128  # 16 t-chunks

    with tile.TileContext(nc) as tc, ExitStack() as ctx:
        pers = ctx.enter_context(tc.tile_pool(name="pers", bufs=1))
        work = ctx.enter_context(tc.tile_pool(name="work", bufs=3))
        rpool = ctx.enter_context(tc.tile_pool(name="rpool", bufs=2))
        ps_st = ctx.enter_context(tc.tile_pool(name="ps_st", bufs=2, space="PSUM"))
        ps_y = ctx.enter_context(tc.tile_pool(name="ps_y", bufs=2, space="PSUM"))
        ps_o = ctx.enter_context(tc.tile_pool(name="ps_o", bufs=2, space="PSUM"))

        # ---- persistent tiles ----
        qaug = [pers.tile([128, T], fr, tag=f"qaug{h}", name=f"qaug{h}") for h in range(HL)]
        kaug = [pers.tile([128, T], fr, tag=f"kaug{h}", name=f"kaug{h}") for h in range(HL)]
        # v tiles: per t-chunk [128, 4 heads, 128]; cols 0:64 = v data, 64:128 = ones
        vaug = [pers.tile([128, HL, 128], fr, tag=f"vaug{t}", name=f"vaug{t}")
                for t in range(NT16)]
        yT = [pers.tile([128, T], fr, tag=f"yT{i}", name=f"yT{i}") for i in range(HL // 2)]
        # consolidated weight/input tiles: C-chunk k is a middle free dim so a
        # single DMA (rearranged DRAM AP) loads all chunks
        xs_t = pers.tile([128, 8, T], fr, tag="xs")
        wqk_t = pers.tile([128, 8, 2 * HD], fr, tag="wqkt")
        wv_t = pers.tile([128, 8, HD], fr, tag="wvt")
        wp_t = pers.tile([128, 2, C], fr, tag="wpt")
        xs = [xs_t[:, k, :] for k in range(8)]
        wqks = [wqk_t[:, k, :] for k in range(8)]
        wvs = [wv_t[:, k, :] for k in range(8)]
        wp_sb = [wp_t[:, i, :] for i in range(2)]
        bqk_sb = pers.tile([1, 2 * HD], fr, tag="bqk")
        bv_sb = pers.tile([1, HD], fr, tag="bv")
        umin = pers.tile([128, 128], fr, tag="umin")
        onesrow = pers.tile([1, IW], fr, tag="onesrow")

        # ---- DMAs ordered by first consumption; consolidated via rearranged
        # DRAM APs (chunk dim k folded into the free dims).  x streams in
        # 512-column quarters so early qkT tiles can start immediately.
        nc.sync.dma_start(bqk_sb[:], bqk[:])
        nc.sync.dma_start(bv_sb[:], bv[:])
        nc.sync.dma_start(onesrow[:], onesr_d[:])
        wqk_r = wqk.rearrange("(k p) c -> p k c", p=128)
        xT_r = xT.rearrange("(k p) t -> p k t", p=128)
        nc.sync.dma_start(wqk_t[:, 0:4, :], wqk_r[:, 0:4, :])
        nc.sync.dma_start(umin[:], umin_d[:])
        for h in range(HL):
            nc.sync.dma_start(qaug[h][64:66, :], aq[2 * h:2 * h + 2, :])
            nc.sync.dma_start(kaug[h][64:66, :], ak[:, :])
        nc.sync.dma_start(
            xs_t[:, :, 0:IW], xT_r[:, :, 0:IW])
        nc.sync.dma_start(wqk_t[:, 4:8, :], wqk_r[:, 4:8, :])
        for tt in range(1, 4):
            nc.sync.dma_start(
                xs_t[:, :, tt * IW:(tt + 1) * IW],
                xT_r[:, :, tt * IW:(tt + 1) * IW])
        nc.sync.dma_start(wv_t[:], wv.rearrange("(k p) c -> p k c", p=128))
        # ones columns of vaug (everything 1.0; v-evac overwrites cols 0:64)
        for t in range(NT16):
            nc.vector.memset(vaug[t][:], 1.0)
        nc.sync.dma_start(wp_t[:], wp.rearrange("(i p) c -> p i c", p=128))

        # ---------------- phase 1 helpers ----------------
        def qk_tile(cc, tt):
            # psum [128, IW] = wqk cols [cc*128:(cc+1)*128] x tokens tt*IW..
            # (shares the outproj pool: phase-1 and outproj never overlap)
            ps = ps_o.tile([128, IW], f32, tag="o", name="qkps")
            for k in range(8):
                nc.tensor.matmul(
                    ps[:], wqks[k][:, cc * 128:(cc + 1) * 128],
                    xs[k][:, tt * IW:(tt + 1) * IW],
                    start=(k == 0), stop=False)
            nc.tensor.matmul(
                ps[:], bqk_sb[0:1, cc * 128:(cc + 1) * 128], onesrow[:],
                start=False, stop=True)
            for half in range(2):
                h = (cc % 2) * 2 + half
                dst = qaug[h] if cc < 2 else kaug[h]
                nc.vector.tensor_copy(
                    dst[0:64, tt * IW:(tt + 1) * IW],
                    ps[half * 64:(half + 1) * 64, :])

        def v_tile(t16):
            ps = ps_o.tile([128, HD], f32, tag="o", name="vps")
            for k in range(8):
                nc.tensor.matmul(
                    ps[:], xs[k][:, t16 * 128:(t16 + 1) * 128], wvs[k][:],
                    start=(k == 0), stop=False)
            nc.tensor.matmul(
                ps[:], onesrow[:, 0:128], bv_sb[:], start=False, stop=True)
            for h in range(HL):
                nc.vector.tensor_copy(
                    vaug[t16][:, h, 0:64], ps[:, h * 64:(h + 1) * 64])

        # ---------------- attention for one (i-tile, head) ----------------
        def attn(it, h):
            i0 = it * IW
            njc = i0 // 128 + IW // 128
            npair = njc // 2
            yacc = ps_y.tile([128, IW], f32, tag="yacc", name="yacc")
            pend = None  # (p tile, widths) of the pair whose PV is not yet emitted

            def emit_pv(p, c0a, c0b, Wa, Wb, pj):
                nc.tensor.matmul(
                    yacc[:, c0a:IW], vaug[2 * pj][:, h, :], p[:, 0:Wa],
                    start=(pj == 0), stop=False)
                nc.tensor.matmul(
                    yacc[:, c0b:IW], vaug[2 * pj + 1][:, h, :], p[:, Wa:Wa + Wb],
                    start=False, stop=(pj == npair - 1))

            for pj in range(npair):
                j0a = (2 * pj) * 128
                j0b = j0a + 128
                c0a = max(0, j0a - i0)
                c0b = max(0, j0b - i0)
                Wa = IW - c0a
                Wb = IW - c0b
                st2 = ps_st.tile([128, 2 * IW], f32, tag="st", name="st")
                nc.tensor.matmul(
                    st2[:, 0:Wa],
                    kaug[h][0:66, j0a:j0a + 128],
                    qaug[h][0:66, i0 + c0a:i0 + IW],
                    start=True, stop=True)
                nc.tensor.matmul(
                    st2[:, Wa:Wa + Wb],
                    kaug[h][0:66, j0b:j0b + 128],
                    qaug[h][0:66, i0 + c0b:i0 + IW],
                    start=True, stop=True)
                if pend is not None:
                    emit_pv(*pend)
                    pend = None
                p = work.tile([128, 2 * IW], fr, tag="p", name="p")
                nc.scalar.activation(p[:, 0:Wa + Wb], st2[:, 0:Wa + Wb], Exp)
                if j0a >= i0:
                    # diagonal squares at p cols [0:128] (chunk a) and
                    # [Wa:Wa+128] (chunk b): min with {30000 keep, 0 drop}
                    nc.vector.tensor_tensor(p[:, 0:128], p[:, 0:128], umin[:], Min)
                    nc.vector.tensor_tensor(
                        p[:, Wa:Wa + 128], p[:, Wa:Wa + 128], umin[:], Min)
                pend = (p, c0a, c0b, Wa, Wb, pj)
            emit_pv(*pend)
            # normalization: rows 64:128 of yacc hold the denominator
            # (broadcast by the 64 ones-columns of vaug).
            den = rpool.tile([64, IW], f32, tag="den", name="den")
            nc.vector.tensor_copy(den[:], yacc[64:128, :])
            rec = rpool.tile([64, IW], f32, tag="rec", name="rec")
            nc.vector.reciprocal_approx_fast(rec[:], den[:])
            nc.vector.tensor_mul(
                yT[h // 2][(h % 2) * 64:(h % 2) * 64 + 64, i0:i0 + IW],
                yacc[0:64, :], rec[:])

        def outproj(it):
            for tp in range(2 * it, 2 * it + 2):  # pairs of t-chunks
                ot = work.tile([128, 2, C], fr, tag="ot", name="ot")
                for half in range(2):
                    t16 = 2 * tp + half
                    for e2 in range(2):
                        ps = ps_o.tile([128, 512], f32, tag="o", name="ops")
                        for kk in range(2):
                            nc.tensor.matmul(
                                ps[:],
                                yT[kk][:, t16 * 128:(t16 + 1) * 128],
                                wp_sb[kk][:, e2 * 512:(e2 + 1) * 512],
                                start=(kk == 0), stop=(kk == 1))
                        nc.vector.tensor_copy(
                            ot[:, half, e2 * 512:(e2 + 1) * 512], ps[:])
                out_r = out[tp * 256:(tp + 1) * 256, :].rearrange(
                    "(a p) c -> p a c", p=128)
                nc.sync.dma_start(out_r, ot[:])

        # ---------------- program order ----------------
        # Fine-grained interleave: each attention block is preceded only by
        # the projection tiles it needs; qk/v/outproj fill PE slack during
        # ACT-bound attention stretches without touching the St psum slots.
        for it in range(4):
            qk_tile(0, it)
            qk_tile(2, it)
            for t16 in range(4 * it, 4 * it + 4):
                v_tile(t16)
            if it > 0:
                outproj(it - 1)
            attn(it, 0)
            attn(it, 1)
            qk_tile(1, it)
            qk_tile(3, it)
            attn(it, 2)
            attn(it, 3)
        outproj(3)

    nc.compile()
    return nc


def _get_nc():
    if "nc" not in _CACHE:
        _CACHE["nc"] = _build_nc()
    return _CACHE["nc"]


def _shard_inputs(x, W_attn, b_attn, W_proj, b_proj):
    f16 = np.float16
    slopes = (1.0 / np.power(2.0, np.arange(1, H + 1))).astype(np.float32)
    iota = np.arange(T, dtype=np.float32)
    ak = np.stack([np.ones(T, np.float32), iota]).astype(f16)      # [2, T]
    # min-mask for diagonal squares: keep (j<=i) -> 30000, drop -> 0
    pp, ff = np.meshgrid(np.arange(128), np.arange(128), indexing="ij")
    umin = np.where(pp <= ff, 30000.0, 0.0).astype(f16)
    xTs = [np.ascontiguousarray(x[b].T).astype(f16) for b in range(x.shape[0])]

    in_maps = []
    for core in range(8):
        b, g = core // 4, core % 4
        cs = slice(g * HD, (g + 1) * HD)
        q_cols = W_attn[:, 0:C][:, cs] * 0.125
        k_cols = W_attn[:, C:2 * C][:, cs]
        v_cols = np.ascontiguousarray(W_attn[:, 2 * C:3 * C][:, cs])
        wqk_l = np.ascontiguousarray(np.concatenate([q_cols, k_cols], axis=1))
        bqk_l = np.concatenate(
            [b_attn[0:C][cs] * 0.125, b_attn[C:2 * C][cs]])[None, :]
        bv_l = b_attn[2 * C:3 * C][cs][None, :]
        wp_l = np.ascontiguousarray(W_proj[g * HD:(g + 1) * HD, :])
        aq = np.zeros((2 * HL, T), np.float32)
        for hh in range(HL):
            s = slopes[g * HL + hh]
            aq[2 * hh, :] = -s * iota
            aq[2 * hh + 1, :] = s
        in_maps.append({
            "xT": xTs[b], "wqk": wqk_l.astype(f16),
            "bqk": np.ascontiguousarray(bqk_l).astype(f16),
            "wv": v_cols.astype(f16), "bv": np.ascontiguousarray(bv_l).astype(f16),
            "wp": wp_l.astype(f16), "aq": aq.astype(f16), "ak": ak,
            "umin": umin,
            "onesr": np.ones((1, IW), f16),
        })
    return in_maps


def kernel(x, W_attn, b_attn, W_proj, b_proj, _trace=False, _tmpdir=None):
    from concourse.bass_utils import run_bass_kernel_spmd

    x = np.asarray(x, dtype=np.float32)
    W_attn = np.asarray(W_attn, dtype=np.float32)
    b_attn = np.asarray(b_attn, dtype=np.float32)
    W_proj = np.asarray(W_proj, dtype=np.float32)
    b_proj = np.asarray(b_proj, dtype=np.float32)

    nc = _get_nc()
    in_maps = _shard_inputs(x, W_attn, b_attn, W_proj, b_proj)
    res = run_bass_kernel_spmd(
        nc, in_maps, core_ids=list(range(8)), trace=_trace, tmpdir=_tmpdir)
    out = np.empty((x.shape[0], T, C), np.float32)
    for b in range(x.shape[0]):
        acc = res.results[4 * b]["out"].astype(np.float32)
        for i in range(1, 4):
            acc += res.results[4 * b + i]["out"].astype(np.float32)
        out[b] = acc + b_proj
    if _trace:
        kernel.last_exec_time_ns = res.exec_time_ns
    return out


# revision 30
# speedup vs baseline: 1.0085x; 1.0085x over previous
"""AlibiCausalSelfAttention on 8 Trainium2 NeuronCores — v2.

Sharding: data-parallel over batch (B=2) x head-parallel over head groups
(16 heads -> 4 groups of 4). Core c handles batch c//4, heads [4*(c%4), 4*(c%4)+4).
Each core computes a partial projection output [T, C] fp16 (W_proj row-sharded);
the host sums the 4 partials per batch in fp32 and adds b_proj.

v2 structural changes vs v1 (221us):
  - qk bias folded into the projection matmul as a 9th K=1 accumulation step
    (lhsT = bias row, rhs = ones row); PSUM->SBUF evacuations become pure
    cast-copies: qk on DVE, v on ACT (idle during phase 1).
  - v tiles store [64 v-dims | 64 ones] per head, so the PV matmul (M=128)
    broadcasts the softmax denominator into PSUM rows 64..127 for free.
    Normalization is then just reciprocal_approx_fast (rows 64:128 -> 0:64
    partition shift) + one tensor_mul. Replaces v1's reciprocal/broadcast-
    matmul/copy chain (~58us DVE -> ~22us).
  - St pair matmuls placed at packed column offsets [0:Wa|Wa:Wa+Wb] so exp
    covers one contiguous trimmed range; exp writes fp16 directly.  Diagonal
    causal masking via a single tensor_tensor MIN against a {30000,0} mask
    (also squashes fp16 exp overflow inf -> 0).
  - Software-pipelined St(n+1) before PV(n) so the in-order PE queue never
    stalls on ACT's exp.
  - Single tile scope; program order interleaves phase 1 (heads 0,1 first),
    attention, and output projection so all engines stay busy and the PE
    HAM clock gate stays warm.
  - fp16 output (halves the store DMA).
"""

import sys

if "/opt/trn_rl_repo" not in sys.path:
    sys.path.insert(0, "/opt/trn_rl_repo")

import numpy as np

T = 2048
C = 1024
H = 16
D = 64
HL = 4          # heads per core
HD = HL * D     # 256 local head dims
IW = 512        # i-tile width

_CACHE = {}


def _build_nc():
    import concourse.mybir as mybir
    import concourse.tile as tile
    from concourse import bacc
    from contextlib import ExitStack

    f32 = mybir.dt.float32
    fr = mybir.dt.float16
    Exp = mybir.ActivationFunctionType.Exp
    Min = mybir.AluOpType.min

    nc = bacc.Bacc("TRN2", target_bir_lowering=False, debug=False, num_devices=8)

    xT = nc.dram_tensor("xT", [C, T], fr, kind="ExternalInput").ap()
    wqk = nc.dram_tensor("wqk", [C, 2 * HD], fr, kind="ExternalInput").ap()
    bqk = nc.dram_tensor("bqk", [1, 2 * HD], fr, kind="ExternalInput").ap()
    wv = nc.dram_tensor("wv", [C, HD], fr, kind="ExternalInput").ap()
    bv = nc.dram_tensor("bv", [1, HD], fr, kind="ExternalInput").ap()
    wp = nc.dram_tensor("wp", [HD, C], fr, kind="ExternalInput").ap()
    aq = nc.dram_tensor("aq", [2 * HL, T], fr, kind="ExternalInput").ap()
    ak = nc.dram_tensor("ak", [2, T], fr, kind="ExternalInput").ap()
    umin_d = nc.dram_tensor("umin", [128, 128], fr, kind="ExternalInput").ap()
    onesr_d = nc.dram_tensor("onesr", [1, IW], fr, kind="ExternalInput").ap()
    out = nc.dram_tensor("out", [T, C], fr, kind="ExternalOutput").ap()

    NT16 = T // 128  # 16 t-chunks

    with tile.TileContext(nc) as tc, ExitStack() as ctx:
        pers = ctx.enter_context(tc.tile_pool(name="pers", bufs=1))
        work = ctx.enter_context(tc.tile_pool(name="work", bufs=3))
        rpool = ctx.enter_context(tc.tile_pool(name="rpool", bufs=2))
        ps_st = ctx.enter_context(tc.tile_pool(name="ps_st", bufs=2, space="PSUM"))
        ps_y = ctx.enter_context(tc.tile_pool(name="ps_y", bufs=2, space="PSUM"))
        ps_o = ctx.enter_context(tc.tile_pool(name="ps_o", bufs=2, space="PSUM"))

        # ---- persistent tiles ----
        qaug = [pers.tile([128, T], fr, tag=f"qaug{h}", name=f"qaug{h}") for h in range(HL)]
        kaug = [pers.tile([128, T], fr, tag=f"kaug{h}", name=f"kaug{h}") for h in range(HL)]
        # v tiles: per t-chunk [128, 4 heads, 128]; cols 0:64 = v data, 64:128 = ones
        vaug = [pers.tile([128, HL, 128], fr, tag=f"vaug{t}", name=f"vaug{t}")
                for t in range(NT16)]
        yT = [pers.tile([128, T], fr, tag=f"yT{i}", name=f"yT{i}") for i in range(HL // 2)]
        # consolidated weight/input tiles: C-chunk k is a middle free dim so a
        # single DMA (rearranged DRAM AP) loads all chunks
        xs_t = pers.tile([128, 8, T], fr, tag="xs")
        wqk_t = pers.tile([128, 8, 2 * HD], fr, tag="wqkt")
        wv_t = pers.tile([128, 8, HD], fr, tag="wvt")
        wp_t = pers.tile([128, 2, C], fr, tag="wpt")
        xs = [xs_t[:, k, :] for k in range(8)]
        wqks = [wqk_t[:, k, :] for k in range(8)]
        wvs = [wv_t[:, k, :] for k in range(8)]
        wp_sb = [wp_t[:, i, :] for i in range(2)]
        bqk_sb = pers.tile([1, 2 * HD], fr, tag="bqk")
        bv_sb = pers.tile([1, HD], fr, tag="bv")
        umin = pers.tile([128, 128], fr, tag="umin")
        onesrow = pers.tile([1, IW], fr, tag="onesrow")

        # ---- DMAs ordered by first consumption; consolidated via rearranged
        # DRAM APs (chunk dim k folded into the free dims).  x streams in
        # 512-column quarters so early qkT tiles can start immediately.
        nc.sync.dma_start(bqk_sb[:], bqk[:])
        nc.sync.dma_start(bv_sb[:], bv[:])
        nc.sync.dma_start(onesrow[:], onesr_d[:])
        wqk_r = wqk.rearrange("(k p) c -> p k c", p=128)
        xT_r = xT.rearrange("(k p) t -> p k t", p=128)
        nc.sync.dma_start(wqk_t[:, 0:4, :], wqk_r[:, 0:4, :])
        # tiny constant loads issue from the (idle) ACT HWDGE queue so they
        # don't consume sync-queue issue slots ahead of the x stream
        nc.scalar.dma_start(umin[:], umin_d[:])
        for h in range(HL):
            nc.scalar.dma_start(qaug[h][64:66, :], aq[2 * h:2 * h + 2, :])
            nc.scalar.dma_start(kaug[h][64:66, :], ak[:, :])
        nc.sync.dma_start(
            xs_t[:, :, 0:IW], xT_r[:, :, 0:IW])
        nc.sync.dma_start(wqk_t[:, 4:8, :], wqk_r[:, 4:8, :])
        for tt in range(1, 4):
            nc.sync.dma_start(
                xs_t[:, :, tt * IW:(tt + 1) * IW],
                xT_r[:, :, tt * IW:(tt + 1) * IW])
        nc.sync.dma_start(wv_t[:], wv.rearrange("(k p) c -> p k c", p=128))
        # ones columns of vaug (everything 1.0; v-evac overwrites cols 0:64)
        for t in range(NT16):
            nc.vector.memset(vaug[t][:], 1.0)
        nc.sync.dma_start(wp_t[:], wp.rearrange("(i p) c -> p i c", p=128))

        # ---------------- phase 1 helpers ----------------
        def qk_tile(cc, tt):
            # psum [128, IW] = wqk cols [cc*128:(cc+1)*128] x tokens tt*IW..
            # (shares the outproj pool: phase-1 and outproj never overlap)
            ps = ps_o.tile([128, IW], f32, tag="o", name="qkps")
            for k in range(8):
                nc.tensor.matmul(
                    ps[:], wqks[k][:, cc * 128:(cc + 1) * 128],
                    xs[k][:, tt * IW:(tt + 1) * IW],
                    start=(k == 0), stop=False)
            nc.tensor.matmul(
                ps[:], bqk_sb[0:1, cc * 128:(cc + 1) * 128], onesrow[:],
                start=False, stop=True)
            for half in range(2):
                h = (cc % 2) * 2 + half
                dst = qaug[h] if cc < 2 else kaug[h]
                nc.vector.tensor_copy(
                    dst[0:64, tt * IW:(tt + 1) * IW],
                    ps[half * 64:(half + 1) * 64, :])

        def v_tile(t16):
            ps = ps_o.tile([128, HD], f32, tag="o", name="vps")
            for k in range(8):
                nc.tensor.matmul(
                    ps[:], xs[k][:, t16 * 128:(t16 + 1) * 128], wvs[k][:],
                    start=(k == 0), stop=False)
            nc.tensor.matmul(
                ps[:], onesrow[:, 0:128], bv_sb[:], start=False, stop=True)
            for h in range(HL):
                nc.vector.tensor_copy(
                    vaug[t16][:, h, 0:64], ps[:, h * 64:(h + 1) * 64])

        # ---------------- attention for one (i-tile, head) ----------------
        def attn(it, h):
            i0 = it * IW
            njc = i0 // 128 + IW // 128
            npair = njc // 2
            yacc = ps_y.tile([128, IW], f32, tag="yacc", name="yacc")
            pend = None  # (p tile, widths) of the pair whose PV is not yet emitted

            def emit_pv(p, c0a, c0b, Wa, Wb, pj):
                nc.tensor.matmul(
                    yacc[:, c0a:IW], vaug[2 * pj][:, h, :], p[:, 0:Wa],
                    start=(pj == 0), stop=False)
                nc.tensor.matmul(
                    yacc[:, c0b:IW], vaug[2 * pj + 1][:, h, :], p[:, Wa:Wa + Wb],
                    start=False, stop=(pj == npair - 1))

            for pj in range(npair):
                j0a = (2 * pj) * 128
                j0b = j0a + 128
                c0a = max(0, j0a - i0)
                c0b = max(0, j0b - i0)
                Wa = IW - c0a
                Wb = IW - c0b
                st2 = ps_st.tile([128, 2 * IW], f32, tag="st", name="st")
                nc.tensor.matmul(
                    st2[:, 0:Wa],
                    kaug[h][0:66, j0a:j0a + 128],
                    qaug[h][0:66, i0 + c0a:i0 + IW],
                    start=True, stop=True)
                nc.tensor.matmul(
                    st2[:, Wa:Wa + Wb],
                    kaug[h][0:66, j0b:j0b + 128],
                    qaug[h][0:66, i0 + c0b:i0 + IW],
                    start=True, stop=True)
                if pend is not None:
                    emit_pv(*pend)
                    pend = None
                p = work.tile([128, 2 * IW], fr, tag="p", name="p")
                nc.scalar.activation(p[:, 0:Wa + Wb], st2[:, 0:Wa + Wb], Exp)
                if j0a >= i0:
                    # diagonal squares at p cols [0:128] (chunk a) and
                    # [Wa:Wa+128] (chunk b): min with {30000 keep, 0 drop}
                    nc.vector.tensor_tensor(p[:, 0:128], p[:, 0:128], umin[:], Min)
                    nc.vector.tensor_tensor(
                        p[:, Wa:Wa + 128], p[:, Wa:Wa + 128], umin[:], Min)
                pend = (p, c0a, c0b, Wa, Wb, pj)
            emit_pv(*pend)
            # normalization: rows 64:128 of yacc hold the denominator
            # (broadcast by the 64 ones-columns of vaug).
            den = rpool.tile([64, IW], f32, tag="den", name="den")
            nc.vector.tensor_copy(den[:], yacc[64:128, :])
            rec = rpool.tile([64, IW], f32, tag="rec", name="rec")
            nc.vector.reciprocal_approx_fast(rec[:], den[:])
            nc.vector.tensor_mul(
                yT[h // 2][(h % 2) * 64:(h % 2) * 64 + 64, i0:i0 + IW],
                yacc[0:64, :], rec[:])

        def outproj(it):
            for tp in range(2 * it, 2 * it + 2):  # pairs of t-chunks
                ot = work.tile([128, 2, C], fr, tag="ot", name="ot")
                for half in range(2):
                    t16 = 2 * tp + half
                    for e2 in range(2):
                        ps = ps_o.tile([128, 512], f32, tag="o", name="ops")
                        for kk in range(2):
                            nc.tensor.matmul(
                                ps[:],
                                yT[kk][:, t16 * 128:(t16 + 1) * 128],
                                wp_sb[kk][:, e2 * 512:(e2 + 1) * 512],
                                start=(kk == 0), stop=(kk == 1))
                        nc.vector.tensor_copy(
                            ot[:, half, e2 * 512:(e2 + 1) * 512], ps[:])
                out_r = out[tp * 256:(tp + 1) * 256, :].rearrange(
                    "(a p) c -> p a c", p=128)
                nc.sync.dma_start(out_r, ot[:])

        # ---------------- program order ----------------
        # HAM warmup: junk matmuls spanning the initial DMA wait keep the PE
        # clock gate at 8/8 so real matmuls run at 2.4 GHz from the start.
        wdum = pers.tile([128, IW], fr, tag="wdum")
        nc.vector.memset(wdum[:], 0.0)
        for w in range(32):
            psd = ps_st.tile([128, IW], f32, tag="st", name="psd")
            nc.tensor.matmul(psd[:], wdum[:, 0:128], wdum[:], start=True, stop=True)
        # Fine-grained interleave: each attention block is preceded only by
        # the projection tiles it needs; qk/v/outproj fill PE slack during
        # ACT-bound attention stretches without touching the St psum slots.
        for it in range(4):
            qk_tile(0, it)
            qk_tile(2, it)
            for t16 in range(4 * it, 4 * it + 4):
                v_tile(t16)
            if it > 0:
                outproj(it - 1)
            attn(it, 0)
            attn(it, 1)
            qk_tile(1, it)
            qk_tile(3, it)
            attn(it, 2)
            attn(it, 3)
        outproj(3)

    nc.compile()
    return nc


def _get_nc():
    if "nc" not in _CACHE:
        _CACHE["nc"] = _build_nc()
    return _CACHE["nc"]


def _shard_inputs(x, W_attn, b_attn, W_proj, b_proj):
    f16 = np.float16
    slopes = (1.0 / np.power(2.0, np.arange(1, H + 1))).astype(np.float32)
    iota = np.arange(T, dtype=np.float32)
    ak = np.stack([np.ones(T, np.float32), iota]).astype(f16)      # [2, T]
    # min-mask for diagonal squares: keep (j<=i) -> 30000, drop -> 0
    pp, ff = np.meshgrid(np.arange(128), np.arange(128), indexing="ij")
    umin = np.where(pp <= ff, 30000.0, 0.0).astype(f16)
    xTs = [np.ascontiguousarray(x[b].T).astype(f16) for b in range(x.shape[0])]

    in_maps = []
    for core in range(8):
        b, g = core // 4, core % 4
        cs = slice(g * HD, (g + 1) * HD)
        q_cols = W_attn[:, 0:C][:, cs] * 0.125
        k_cols = W_attn[:, C:2 * C][:, cs]
        v_cols = np.ascontiguousarray(W_attn[:, 2 * C:3 * C][:, cs])
        wqk_l = np.ascontiguousarray(np.concatenate([q_cols, k_cols], axis=1))
        bqk_l = np.concatenate(
            [b_attn[0:C][cs] * 0.125, b_attn[C:2 * C][cs]])[None, :]
        bv_l = b_attn[2 * C:3 * C][cs][None, :]
        wp_l = np.ascontiguousarray(W_proj[g * HD:(g + 1) * HD, :])
        aq = np.zeros((2 * HL, T), np.float32)
        for hh in range(HL):
            s = slopes[g * HL + hh]
            aq[2 * hh, :] = -s * iota
            aq[2 * hh + 1, :] = s
        in_maps.append({
            "xT": xTs[b], "wqk": wqk_l.astype(f16),
            "bqk": np.ascontiguousarray(bqk_l).astype(f16),
            "wv": v_cols.astype(f16), "bv": np.ascontiguousarray(bv_l).astype(f16),
            "wp": wp_l.astype(f16), "aq": aq.astype(f16), "ak": ak,
            "umin": umin,
            "onesr": np.ones((1, IW), f16),
        })
    return in_maps


def kernel(x, W_attn, b_attn, W_proj, b_proj, _trace=False, _tmpdir=None):
    from concourse.bass_utils import run_bass_kernel_spmd

    x = np.asarray(x, dtype=np.float32)
    W_attn = np.asarray(W_attn, dtype=np.float32)
    b_attn = np.asarray(b_attn, dtype=np.float32)
    W_proj = np.asarray(W_proj, dtype=np.float32)
    b_proj = np.asarray(b_proj, dtype=np.float32)

    nc = _get_nc()
    in_maps = _shard_inputs(x, W_attn, b_attn, W_proj, b_proj)
    res = run_bass_kernel_spmd(
        nc, in_maps, core_ids=list(range(8)), trace=_trace, tmpdir=_tmpdir)
    out = np.empty((x.shape[0], T, C), np.float32)
    for b in range(x.shape[0]):
        acc = res.results[4 * b]["out"].astype(np.float32)
        for i in range(1, 4):
            acc += res.results[4 * b + i]["out"].astype(np.float32)
        out[b] = acc + b_proj
    if _trace:
        kernel.last_exec_time_ns = res.exec_time_ns
    return out
